# revision 25
# baseline (speedup 1.0000x reference)
"""Trainium2 Bass kernel for the CGIM sparse-attention block.

Per-sample math (reference):
  Qf = Wq @ [F1;F2] + bq            (1x1 conv, transposed-layout on device)
  Qs = softmax_d(Qf per head)
  per branch i: K = Wk_i @ F_i (+bk_i cancels), V = Wv_i @ F_i + bv_i
                Ks = softmax_hw(K);  Att = Ks @ Qs;  Xw = Att @ V
  fused = concat(mu*X1 + F1, mu*X2 + F2)
  y = relu(BN(conv3x3(fused, Wc)))

Sharding: data-parallel over batch (B=8) across the 8 NeuronCores; weights
replicated. Each core computes one sample end to end; no collectives.

Key device-side algebra:
 - K softmax bias cancels (constant along softmax axis) -> dropped.
 - K softmax denominator S_d is postponed all the way to the Xw epilogue,
   where d is the partition axis (fused scale mu/S_d).
 - Q/K computed directly in transposed [hw, c] layout (stationary = F tile),
   so Q's per-head softmax is a free-dim segmented reduce and the Att
   contraction over hw needs no transposes at all.
 - AttT computed directly as lhsT for the Xw matmul; only the 32x32
   diagonal (per-head) blocks are copied into a zeroed block-diag tile.
 - conv3x3 = 36 accumulated shifted 1x1 matmuls over a zero-padded
   [66 x 72] image layout; BN+ReLU folded into the PSUM->SBUF ACT.
All matmul operands bf16, fp32 PSUM accumulation.

Scheduling notes (perf):
 - Q-bias and K-colsum matmuls use a full 128x128 ones matrix as lhsT:
   1-row/1-col stationaries force row_grp/col_grp masks and each mask
   switch costs ~+100ns on the next matmul.
 - Dummy warmup matmuls run during the DMA preamble so the PE HAM clock
   gate is already at 8/8 when phase 1 starts.
 - Xw chunks are emitted n8-major and the conv3x3 is emitted per-output-
   chunk (36-matmul accumulation groups) interleaved with them, so the
   PE never waits on the Vector/GpSimd epilogue and the conv tail
   (ACT+DMA) pipelines under the next group's matmuls.
 - The fused-residual epilogue alternates Vector / GpSimd so neither
   paces the PE.
"""

import numpy as np
import ml_dtypes

import concourse.bass as bass
import concourse.mybir as mybir
import concourse.tile as tile
from concourse import bacc
from concourse.bass_utils import run_bass_kernel_spmd

BF16 = mybir.dt.bfloat16
F32 = mybir.dt.float32
F8 = mybir.dt.float8e4
DR = mybir.MatmulPerfMode.DoubleRow
AF = mybir.ActivationFunctionType
ALU = mybir.AluOpType
AX = mybir.AxisListType

B, C, H, W = 8, 256, 64, 64
HW = H * W                  # 4096
NH, D = 8, 32               # heads, per-head dim
NT = HW // 128              # 32 hw-tiles of 128
PH, PW = H + 2, 72          # padded conv image (66 rows x 72 cols)
N_CORES = 8
BN_EPS = 1e-5

_bf = ml_dtypes.bfloat16
_f8 = ml_dtypes.float8_e4m3fn


def _build_program() -> bass.Bass:
    nc = bacc.Bacc("TRN2", target_bir_lowering=False)

    # ---- DRAM I/O (per core) ----
    f1_d = nc.dram_tensor("f1", [C, HW], BF16, kind="ExternalInput").ap()
    f2_d = nc.dram_tensor("f2", [C, HW], BF16, kind="ExternalInput").ap()
    f8_d = nc.dram_tensor("f8", [128, 2, 2, HW], F8, kind="ExternalInput").ap()
    wq8_d = nc.dram_tensor("wq8", [128, 2, 2, 256], F8, kind="ExternalInput").ap()
    wk8_d = nc.dram_tensor("wk8", [128, 2, 2, 256], F8, kind="ExternalInput").ap()
    wv8_d = nc.dram_tensor("wv8", [128, 2, 2, 256], F8, kind="ExternalInput").ap()
    wc_d = nc.dram_tensor("wc", [128, 4, 18, 128], BF16, kind="ExternalInput").ap()
    bqr_d = nc.dram_tensor("bqr", [128, 256], BF16, kind="ExternalInput").ap()
    bv_d = nc.dram_tensor("bv", [128, 2, 2], F32, kind="ExternalInput").ap()
    bns_d = nc.dram_tensor("bns", [128, 2], F32, kind="ExternalInput").ap()
    bnb_d = nc.dram_tensor("bnb", [128, 2], F32, kind="ExternalInput").ap()
    muv_d = nc.dram_tensor("muv", [128, 1], F32, kind="ExternalInput").ap()
    y_d = nc.dram_tensor("y", [C, HW], F32, kind="ExternalOutput").ap()

    with tile.TileContext(nc) as tc:
        with tc.tile_pool(name="per", bufs=1) as per, \
             tc.tile_pool(name="sml", bufs=4) as sml:

            # ---- persistent SBUF tiles ----
            f8t = per.tile([128, 2, 2, HW], F8)
            wq8 = per.tile([128, 2, 2, 256], F8)
            wk8 = per.tile([128, 2, 2, 256], F8)
            wv8 = per.tile([128, 2, 2, 256], F8)
            wc = per.tile([128, 4, 18, 128], BF16)
            bqr = per.tile([128, 256], BF16)
            bv = per.tile([128, 2, 2], F32)
            bns = per.tile([128, 2], F32)
            bnb = per.tile([128, 2], F32)
            muv = per.tile([128, 1], F32)

            ones = per.tile([128, 128], BF16)
            nc.vector.memset(ones, 1.0)

            fbf = per.tile([128, 4, HW], BF16)        # [F1;F2] as 4 ci-tiles
            # qk[:, n, 0:256]=exp(QfT) (normalized in place),
            # [:, n, 256:512]=exp(K1fT), [:, n, 512:768]=exp(K2fT)
            qk = per.tile([128, NT, 768], BF16)
            vsb1 = per.tile([128, 2, HW], BF16)       # V1, 2 m-groups
            vsb2 = per.tile([128, 2, HW], BF16)
            fp = [per.tile([128, PH, PW], BF16, tag=f"fp{j}", name=f"fp{j}")
                  for j in range(4)]
            attbd = [per.tile([128, 128], BF16, tag=f"abd{j}", name=f"abd{j}")
                     for j in range(4)]

            # PE warmup: dummy matmuls during the DMA preamble keep the PE
            # busy through the HAM window so phase 1 starts at 2.4 GHz.
            warm_ctx = tc.tile_pool(name="warm", bufs=1, space="PSUM")
            warm = warm_ctx.__enter__()
            wt = warm.tile([128, 64], F32)
            for _ in range(25):
                nc.tensor.matmul(wt, ones, ones[:, 0:64],
                                 start=True, stop=True)
            warm_ctx.__exit__(None, None, None)

            # ---- preamble DMA: first-needed tiles fan out across queues ----
            # DMA *issue* costs ~0.65us per dma_start per sequencer, so the
            # first phase-1 deps go one-per-queue in consumption order, then
            # bulk loads follow.
            nc.sync.dma_start(f8t[:, 0, :, 0:128], f8_d[:, 0, :, 0:128])
            nc.gpsimd.dma_start(f8t[:, 1, :, 0:128], f8_d[:, 1, :, 0:128])
            nc.scalar.dma_start(wq8, wq8_d)
            nc.scalar.dma_start(wk8, wk8_d)
            nc.scalar.dma_start(bqr, bqr_d)

            # bulk fp8 F: consumption-ordered ranges; phase 1 reads only
            # this 2MB copy so the early DMA window is half as deep as the
            # bf16 residual copy (which is deferred to mid-phase-1).
            bounds = [128, 512, 1024, 2048, 3072, 4096]
            rr = [nc.gpsimd, nc.sync, nc.gpsimd, nc.sync,
                  nc.gpsimd, nc.sync, nc.gpsimd, nc.sync,
                  nc.gpsimd, nc.sync]
            k = 0
            for ch in range(len(bounds) - 1):
                lo, hi = bounds[ch], bounds[ch + 1]
                for h in range(2):
                    rr[k].dma_start(f8t[:, h, :, lo:hi],
                                    f8_d[:, h, :, lo:hi])
                    k += 1

            # zero conv-halo borders + attbd (stt / block copies fill the
            # interior; only rows 0,65 and cols 0,65 must be zero).
            for j in range(4):
                nc.vector.memset(fp[j][:, 0:1, :], 0.0)
                nc.vector.memset(fp[j][:, 65:66, :], 0.0)
                nc.vector.memset(fp[j][:, :, 0:1], 0.0)
                nc.vector.memset(fp[j][:, :, 65:66], 0.0)
                nc.vector.memset(attbd[j], 0.0)

            # ================= Phase 1: transposed Q/K1/K2 + softmax pieces
            # One [128,768] PSUM tile spans 2 banks: Q(0:256)+K1(256:512) in
            # bank A, K2(512:768) in bank B. One start/stop per bank; other
            # first-writes rely on per-element has_written. One ACT does all
            # three exps.
            pv_ctx = tc.tile_pool(name="pv", bufs=3, space="PSUM")
            pv = pv_ctx.__enter__()
            with tc.tile_pool(name="pq", bufs=2, space="PSUM") as pq:

                gk = dict(skip_group_check=True)
                for n in range(NT):
                    pqk = pq.tile([128, 768], F32, tag="qk")
                    psq, psk1, psk2 = pqk[:, 0:256], pqk[:, 256:512], pqk[:, 512:768]
                    for h in range(2):
                        lhsT8 = f8t[:, h, :, n * 128:(n + 1) * 128]
                        nc.tensor.matmul(psq, lhsT8, wq8[:, h],
                                         start=(h == 0), stop=False,
                                         perf_mode=DR, **gk)
                        if h == 0:
                            nc.tensor.matmul(psk1, lhsT8, wk8[:, 0],
                                             start=False, stop=False,
                                             perf_mode=DR, **gk)
                        else:
                            nc.tensor.matmul(psk2, lhsT8, wk8[:, 1],
                                             start=True, stop=True,
                                             perf_mode=DR, **gk)
                    # bias: full-array rank-128 matmul (ones.T @ (bq/128 rows))
                    nc.tensor.matmul(psq, ones, bqr, start=False, stop=True, **gk)

                    act = nc.scalar.activation(qk[:, n, :], pqk, AF.Exp)
                    if n == 2:
                        wv_anchor = act
                    if n == 8:
                        wc_anchor = act
                    if n == 16:
                        fbf_anchor = act

                    # per-head softmax denominator + normalize (in place)
                    q3 = qk[:, n, 0:256].rearrange("p (h e) -> p h e", h=NH)
                    rq = sml.tile([128, NH], F32, tag="rq")
                    nc.vector.tensor_reduce(rq, q3, axis=AX.X, op=ALU.add)
                    rr = sml.tile([128, NH], F32, tag="rr")
                    nc.vector.reciprocal(rr, rq)
                    # alternate engines so Vector doesn't pace the phase
                    meng = nc.vector if n % 2 == 0 else nc.gpsimd
                    meng.tensor_mul(q3, q3, rr.to_broadcast([128, NH, D]))

            # wv/wc: start loading mid-phase-1 (nosync dep keeps them out of
            # the preamble DMA window where F supply is the bottleneck).
            from concourse.tile import add_dep_helper
            d = nc.sync.dma_start(wv8, wv8_d)
            add_dep_helper(d.ins, wv_anchor.ins, sync=False,
                           reason="defer wv load past preamble")
            f_src = [f1_d, f1_d, f2_d, f2_d]
            fb_bounds = [0, 1024, 2048, 3072, 4096]
            fb_rr = [nc.gpsimd, nc.sync, nc.scalar, nc.gpsimd,
                     nc.sync, nc.scalar, nc.gpsimd, nc.sync,
                     nc.gpsimd, nc.sync, nc.gpsimd, nc.sync,
                     nc.gpsimd, nc.sync, nc.gpsimd, nc.sync]
            k = 0
            for ch in range(len(fb_bounds) - 1):
                lo, hi = fb_bounds[ch], fb_bounds[ch + 1]
                for ci in range(4):
                    half = (ci % 2) * 128
                    d = fb_rr[k].dma_start(fbf[:, ci, lo:hi],
                                           f_src[ci][half:half + 128, lo:hi])
                    add_dep_helper(d.ins, fbf_anchor.ins, sync=False,
                                   reason="defer bf16 residual F load past "
                                          "the fp8 phase-1 supply window")
                    k += 1
            d = nc.sync.dma_start(bv, bv_d)
            add_dep_helper(d.ins, wv_anchor.ins, sync=False, reason="defer")
            d = nc.sync.dma_start(muv, muv_d)
            add_dep_helper(d.ins, wv_anchor.ins, sync=False, reason="defer")
            for ci in range(4):
                d = nc.sync.dma_start(wc[:, ci, :, :], wc_d[:, ci, :, :])
                add_dep_helper(d.ins, wc_anchor.ins, sync=False,
                               reason="defer wc load past preamble")
            d = nc.sync.dma_start(bns, bns_d)
            add_dep_helper(d.ins, wc_anchor.ins, sync=False, reason="defer")
            d = nc.sync.dma_start(bnb, bnb_d)
            add_dep_helper(d.ins, wc_anchor.ins, sync=False, reason="defer")

            # ================= Phase 2: V convs + AttT + block-diag
            def emit_v(pv, br, vsb, fci0):
                # evacuation alternates Scalar/Vector: one engine alone
                # (~700ns per [128,512] chunk) would pace the 432ns matmuls
                for m in range(2):
                    for n8 in range(8):
                        psv = pv.tile([128, 512], F32, tag="v",
                                      name=f"psv{br}{m}{n8}")
                        nc.tensor.matmul(
                            psv, wv8[:, br, :, m * 128:(m + 1) * 128],
                            f8t[:, br, :, n8 * 512:(n8 + 1) * 512],
                            start=True, stop=True, perf_mode=DR)
                        dst = vsb[:, m, n8 * 512:(n8 + 1) * 512]
                        if n8 % 2 == 0:
                            nc.scalar.activation(dst, psv, AF.Identity,
                                                 bias=bv[:, br, m:m + 1])
                        else:
                            nc.vector.tensor_scalar_add(dst, psv,
                                                        bv[:, br, m:m + 1])

            emit_v(pv, 0, vsb1, 0)

            scale = {}
            with tc.tile_pool(name="pa", bufs=4, space="PSUM") as pa:
                psa = {}
                for g in range(2):
                    for br in range(2):
                        p = pa.tile([128, 128], F32, tag="a", name=f"psa{br}{g}")
                        psa[(br, g)] = p

                def copy_group(br, g):
                    # diag blocks into the pre-zeroed block-diag tile
                    p = psa[(br, g)]
                    t = attbd[2 * br + g]
                    for hb in range(4):
                        hs = slice(hb * 32, (hb + 1) * 32)
                        nc.any.tensor_copy(t[hs, hs], p[hs, hs])

                def scale_group(br, g):
                    # K-softmax denominators for free: Qs head-rows sum to
                    # 1, so the column sums of each diagonal Att^T block
                    # are exactly S_d = sum_hw exp(K)[hw, d]. One N=1
                    # matmul replaces 32 [128,512] column-sum matmuls.
                    pS = pa.tile([128, 128], F32, tag="a", name=f"ss{br}{g}")
                    pS = pS[:, 0:1]
                    nc.tensor.matmul(pS, attbd[2 * br + g], ones[:, 0:1],
                                     start=True, stop=True)
                    col = sml.tile([128, 1], F32, tag="scat")
                    nc.vector.reciprocal(col, pS)
                    sc = sml.tile([128, 1], F32, tag="scale")
                    nc.vector.tensor_mul(sc, col, muv)   # mu / S_d
                    scale[(br, g)] = sc

                for g in range(2):
                    for n in range(NT):
                        lhsT = qk[:, n, g * 128:(g + 1) * 128]
                        for br in range(2):
                            nc.tensor.matmul(
                                psa[(br, g)], lhsT,
                                qk[:, n, 256 + br * 256 + g * 128:
                                   256 + br * 256 + (g + 1) * 128],
                                start=(n == 0), stop=(n == NT - 1))
                    # g-group done: copies go out first (Vector), V2 fills
                    # the PE while they run, then the tiny S matmuls
                    copy_group(0, g)
                    copy_group(1, g)
                    if g == 0:
                        emit_v(pv, 1, vsb2, 2)
                    scale_group(0, g)
                    scale_group(1, g)
            pv_ctx.__exit__(None, None, None)

            # ================= Phase 2b/3: Xw + fused epilogue, interleaved
            # with the conv3x3 output chunks. Xw chunks emit n8-major so all
            # four fp tiles grow top-down together; conv group k (output
            # rows 8k..8k+7) needs epilogue rows n8<=k+1 only. The epilogue
            # alternates Vector/GpSimd so the PE stream never waits on it.
            stt_cnt = 0

            def emit_xw_chunk(px, br, g, n8):
                nonlocal stt_cnt
                vsb, fci0 = ((vsb1, 0), (vsb2, 2))[br]
                pxt = px.tile([128, 512], F32, tag="x")
                nc.tensor.matmul(
                    pxt, attbd[2 * br + g],
                    vsb[:, g, n8 * 512:(n8 + 1) * 512],
                    start=True, stop=True)
                # fused = (Xw_raw * mu/S_d) + F  -> padded layout
                j = 2 * br + g
                out = fp[j][:, 1 + n8 * 8:9 + n8 * 8, 1:65]
                fres = fbf[:, fci0 + g, n8 * 512:(n8 + 1) * 512]
                if stt_cnt % 2 == 0:
                    nc.vector.scalar_tensor_tensor(
                        out=out, in0=pxt, scalar=scale[(br, g)], in1=fres,
                        op0=ALU.mult, op1=ALU.add)
                else:
                    # GPSIMD can't read PSUM: Scalar evacuates+scales, then
                    # GpSimd does the all-SBUF residual add.
                    xt = sml.tile([128, 512], BF16, tag="xt")
                    nc.scalar.activation(xt, pxt, AF.Identity,
                                         scale=scale[(br, g)])
                    nc.gpsimd.tensor_add(out, xt, fres)
                stt_cnt += 1

            def emit_conv_group(pc, kk, m, dma_i):
                pst = pc.tile([128, 512], F32, tag="c", name=f"psc{kk}{m}")
                first, last = (0, 0, 0), (3, 2, 2)
                for ci in range(4):
                    for dy in range(3):
                        for dx in range(3):
                            nc.tensor.matmul(
                                pst, wc[:, ci, (dy * 3 + dx) * 2 + m, :],
                                fp[ci][:, kk * 8 + dy:kk * 8 + dy + 8,
                                       dx:dx + 64],
                                start=((ci, dy, dx) == first),
                                stop=((ci, dy, dx) == last))
                ysb = sml.tile([128, 512], F32, tag="y")
                nc.scalar.activation(ysb, pst, AF.Relu,
                                     bias=bnb[:, m:m + 1],
                                     scale=bns[:, m:m + 1])
                eng = nc.sync if dma_i % 2 == 0 else nc.gpsimd
                eng.dma_start(
                    y_d[m * 128:(m + 1) * 128, kk * 512:(kk + 1) * 512], ysb)

            with tc.tile_pool(name="px", bufs=6, space="PSUM") as px, \
                 tc.tile_pool(name="pc", bufs=2, space="PSUM") as pc:
                conv_done = 0
                for n8 in range(8):
                    for br in range(2):
                        for g in range(2):
                            emit_xw_chunk(px, br, g, n8)
                    # conv group k reads fp rows from epilogue chunks
                    # n8 in {k-1, k, k+1}: emit it only after chunk k+1
                    while conv_done < n8:
                        kk = conv_done
                        emit_conv_group(pc, kk, 0, 2 * kk)
                        emit_conv_group(pc, kk, 1, 2 * kk + 1)
                        conv_done += 1
                while conv_done < 8:
                    kk = conv_done
                    emit_conv_group(pc, kk, 0, 2 * kk)
                    emit_conv_group(pc, kk, 1, 2 * kk + 1)
                    conv_done += 1
    nc.compile()
    return nc


_PROGRAM = None


def _get_program():
    global _PROGRAM
    if _PROGRAM is None:
        _PROGRAM = _build_program()
    return _PROGRAM


def kernel(F1, F2, Wq, bq, Wk1, bk1, Wv1, bv1, Wk2, bk2, Wv2, bv2,
           mu, Wc, gamma, beta, rmean, rvar):
    import os
    import sys
    if "antenv.axon_hooks" not in sys.modules:
        try:
            import antenv.axon_hooks  # noqa: F401
        except ImportError:
            # no profiling hook available: make sure a stray BASS_TRACE
            # can't route run_bass_kernel_spmd into the hook import
            os.environ["BASS_NEVER_TRACE"] = "1"
    f32 = np.float32
    F1 = np.asarray(F1, f32)
    F2 = np.asarray(F2, f32)

    def tile_T(w):   # [O, Cin] -> [128, Cin//128, O] (lhsT tiles)
        wt = np.ascontiguousarray(np.asarray(w, f32).T)      # [Cin, O]
        cin, o = wt.shape
        return wt.reshape(cin // 128, 128, o).transpose(1, 0, 2).astype(_bf)

    def tile_Tf(w):  # like tile_T but fp32 (for fp8 cast)
        wt = np.ascontiguousarray(np.asarray(w, f32).T)
        cin, o = wt.shape
        return wt.reshape(cin // 128, 128, o).transpose(1, 0, 2)

    wq8_h = np.ascontiguousarray(
        tile_Tf(Wq).reshape(128, 2, 2, 256)).astype(_f8)
    wk8_h = np.ascontiguousarray(
        np.stack([tile_Tf(Wk1), tile_Tf(Wk2)], axis=1)).astype(_f8)
    wv8_h = np.ascontiguousarray(
        np.stack([tile_Tf(Wv1), tile_Tf(Wv2)], axis=1)).astype(_f8)

    Wc = np.asarray(Wc, f32)                                 # [256, 512, 3, 3]
    # wc[p, ci, (dy*3+dx)*2+m, col] = Wc[m*128+col, ci*128+p, dy, dx]
    wc_h = Wc.reshape(2, 128, 4, 128, 3, 3)                  # m,col,ci,p,dy,dx
    wc_h = wc_h.transpose(3, 2, 4, 5, 0, 1)                  # p,ci,dy,dx,m,col
    wc_h = np.ascontiguousarray(
        wc_h.reshape(128, 4, 18, 128)).astype(_bf)

    bqr_h = np.ascontiguousarray(
        np.tile((np.asarray(bq, f32) / 128.0).reshape(1, 256),
                (128, 1))).astype(_bf)
    # bv_h[p, br, m] = bv_br[m*128 + p]
    bv_h = np.ascontiguousarray(
        np.stack([np.asarray(bv1, f32), np.asarray(bv2, f32)],
                 axis=0).reshape(2, 2, 128).transpose(2, 0, 1))
    inv = np.asarray(gamma, f32) / np.sqrt(np.asarray(rvar, f32) + BN_EPS)
    b2 = np.asarray(beta, f32) - np.asarray(rmean, f32) * inv
    bns_h = np.ascontiguousarray(inv.reshape(2, 128).T)      # [128, 2]
    bnb_h = np.ascontiguousarray(b2.reshape(2, 128).T)
    muv_h = np.full((128, 1), np.asarray(mu, f32).reshape(-1)[0], f32)

    shared = dict(wq8=wq8_h, wk8=wk8_h, wv8=wv8_h, wc=wc_h, bqr=bqr_h,
                  bv=bv_h, bns=bns_h, bnb=bnb_h, muv=muv_h)

    def f8_pack(b):   # [128, 2(h), 2(ko), HW]: cin = 256h + 128ko + p
        cat = np.concatenate([F1[b].reshape(C, HW), F2[b].reshape(C, HW)])
        return np.ascontiguousarray(
            cat.reshape(2, 2, 128, HW).transpose(2, 0, 1, 3)).astype(_f8)

    in_maps = [dict(f1=np.ascontiguousarray(F1[b].reshape(C, HW)).astype(_bf),
                    f2=np.ascontiguousarray(F2[b].reshape(C, HW)).astype(_bf),
                    f8=f8_pack(b),
                    **shared) for b in range(N_CORES)]

    nc = _get_program()
    res = run_bass_kernel_spmd(nc, in_maps, list(range(N_CORES)))
    kernel.last_results = res

    out = np.stack([res.results[b]["y"] for b in range(N_CORES)])
    return out.reshape(B, C, H, W)


kernel.last_results = None


# revision 27
# speedup vs baseline: 1.0208x; 1.0208x over previous
"""Trainium2 Bass kernel for the CGIM sparse-attention block.

Per-sample math (reference):
  Qf = Wq @ [F1;F2] + bq            (1x1 conv, transposed-layout on device)
  Qs = softmax_d(Qf per head)
  per branch i: K = Wk_i @ F_i (+bk_i cancels), V = Wv_i @ F_i + bv_i
                Ks = softmax_hw(K);  Att = Ks @ Qs;  Xw = Att @ V
  fused = concat(mu*X1 + F1, mu*X2 + F2)
  y = relu(BN(conv3x3(fused, Wc)))

Sharding: data-parallel over batch (B=8) across the 8 NeuronCores; weights
replicated. Each core computes one sample end to end; no collectives.

Key device-side algebra:
 - K softmax bias cancels (constant along softmax axis) -> dropped.
 - K softmax denominator S_d is postponed all the way to the Xw epilogue,
   where d is the partition axis (fused scale mu/S_d).
 - Q/K computed directly in transposed [hw, c] layout (stationary = F tile),
   so Q's per-head softmax is a free-dim segmented reduce and the Att
   contraction over hw needs no transposes at all.
 - AttT computed directly as lhsT for the Xw matmul; only the 32x32
   diagonal (per-head) blocks are copied into a zeroed block-diag tile.
 - conv3x3 = 36 accumulated shifted 1x1 matmuls over a zero-padded
   [66 x 72] image layout; BN+ReLU folded into the PSUM->SBUF ACT.
All matmul operands bf16, fp32 PSUM accumulation.

Scheduling notes (perf):
 - Q-bias and K-colsum matmuls use a full 128x128 ones matrix as lhsT:
   1-row/1-col stationaries force row_grp/col_grp masks and each mask
   switch costs ~+100ns on the next matmul.
 - Dummy warmup matmuls run during the DMA preamble so the PE HAM clock
   gate is already at 8/8 when phase 1 starts.
 - Xw chunks are emitted n8-major and the conv3x3 is emitted per-output-
   chunk (36-matmul accumulation groups) interleaved with them, so the
   PE never waits on the Vector/GpSimd epilogue and the conv tail
   (ACT+DMA) pipelines under the next group's matmuls.
 - The fused-residual epilogue alternates Vector / GpSimd so neither
   paces the PE.
"""

import numpy as np
import ml_dtypes

import concourse.bass as bass
import concourse.mybir as mybir
import concourse.tile as tile
from concourse import bacc
from concourse.bass_utils import run_bass_kernel_spmd

BF16 = mybir.dt.bfloat16
F32 = mybir.dt.float32
F8 = mybir.dt.float8e4
DR = mybir.MatmulPerfMode.DoubleRow
AF = mybir.ActivationFunctionType
ALU = mybir.AluOpType
AX = mybir.AxisListType

B, C, H, W = 8, 256, 64, 64
HW = H * W                  # 4096
NH, D = 8, 32               # heads, per-head dim
NT = HW // 128              # 32 hw-tiles of 128
PH, PW = H + 2, 72          # padded conv image (66 rows x 72 cols)
N_CORES = 8
BN_EPS = 1e-5

_bf = ml_dtypes.bfloat16
_f8 = ml_dtypes.float8_e4m3fn


def _build_program() -> bass.Bass:
    nc = bacc.Bacc("TRN2", target_bir_lowering=False)

    # ---- DRAM I/O (per core) ----
    f1_d = nc.dram_tensor("f1", [C, HW], BF16, kind="ExternalInput").ap()
    f2_d = nc.dram_tensor("f2", [C, HW], BF16, kind="ExternalInput").ap()
    f8_d = nc.dram_tensor("f8", [128, 2, 2, HW], F8, kind="ExternalInput").ap()
    wq8_d = nc.dram_tensor("wq8", [128, 2, 2, 256], F8, kind="ExternalInput").ap()
    wk8_d = nc.dram_tensor("wk8", [128, 2, 2, 256], F8, kind="ExternalInput").ap()
    wv8_d = nc.dram_tensor("wv8", [128, 2, 2, 256], F8, kind="ExternalInput").ap()
    wc_d = nc.dram_tensor("wc", [128, 4, 18, 128], BF16, kind="ExternalInput").ap()
    bqr_d = nc.dram_tensor("bqr", [128, 256], BF16, kind="ExternalInput").ap()
    bv_d = nc.dram_tensor("bv", [128, 2, 2], F32, kind="ExternalInput").ap()
    bns_d = nc.dram_tensor("bns", [128, 2], F32, kind="ExternalInput").ap()
    bnb_d = nc.dram_tensor("bnb", [128, 2], F32, kind="ExternalInput").ap()
    muv_d = nc.dram_tensor("muv", [128, 1], F32, kind="ExternalInput").ap()
    y_d = nc.dram_tensor("y", [C, HW], F32, kind="ExternalOutput").ap()

    with tile.TileContext(nc) as tc:
        with tc.tile_pool(name="per", bufs=1) as per, \
             tc.tile_pool(name="sml", bufs=4) as sml:

            # ---- persistent SBUF tiles ----
            f8t = per.tile([128, 2, 2, HW], F8)
            wq8 = per.tile([128, 2, 2, 256], F8)
            wk8 = per.tile([128, 2, 2, 256], F8)
            wv8 = per.tile([128, 2, 2, 256], F8)
            wc = per.tile([128, 4, 18, 128], BF16)
            bqr = per.tile([128, 256], BF16)
            bv = per.tile([128, 2, 2], F32)
            bns = per.tile([128, 2], F32)
            bnb = per.tile([128, 2], F32)
            muv = per.tile([128, 1], F32)

            ones = per.tile([128, 128], BF16)
            nc.vector.memset(ones, 1.0)

            fbf = per.tile([128, 4, HW], BF16)        # [F1;F2] as 4 ci-tiles
            # qk[:, n, 0:256]=exp(QfT) (normalized in place),
            # [:, n, 256:512]=exp(K1fT), [:, n, 512:768]=exp(K2fT)
            qk = per.tile([128, NT, 768], BF16)
            vsb1 = per.tile([128, 2, HW], BF16)       # V1, 2 m-groups
            vsb2 = per.tile([128, 2, HW], BF16)
            fp = [per.tile([128, PH, PW], BF16, tag=f"fp{j}", name=f"fp{j}")
                  for j in range(4)]
            attbd = [per.tile([128, 128], BF16, tag=f"abd{j}", name=f"abd{j}")
                     for j in range(4)]

            # PE warmup: dummy matmuls during the DMA preamble keep the PE
            # busy through the HAM window so phase 1 starts at 2.4 GHz.
            warm_ctx = tc.tile_pool(name="warm", bufs=1, space="PSUM")
            warm = warm_ctx.__enter__()
            wt = warm.tile([128, 64], F32)
            for _ in range(25):
                nc.tensor.matmul(wt, ones, ones[:, 0:64],
                                 start=True, stop=True)
            warm_ctx.__exit__(None, None, None)

            # ---- preamble DMA: first-needed tiles fan out across queues ----
            # DMA *issue* costs ~0.65us per dma_start per sequencer, so the
            # first phase-1 deps go one-per-queue in consumption order, then
            # bulk loads follow.
            nc.sync.dma_start(f8t[:, 0, :, 0:128], f8_d[:, 0, :, 0:128])
            nc.gpsimd.dma_start(f8t[:, 1, :, 0:128], f8_d[:, 1, :, 0:128])
            nc.scalar.dma_start(wq8, wq8_d)
            nc.scalar.dma_start(wk8, wk8_d)
            nc.scalar.dma_start(bqr, bqr_d)

            # bulk fp8 F: consumption-ordered ranges; phase 1 reads only
            # this 2MB copy so the early DMA window is half as deep as the
            # bf16 residual copy (which is deferred to mid-phase-1).
            bounds = [128, 1024, 2048, 3072, 4096]
            rr = [nc.gpsimd, nc.sync, nc.gpsimd, nc.sync,
                  nc.gpsimd, nc.sync, nc.gpsimd, nc.sync]
            k = 0
            for ch in range(len(bounds) - 1):
                lo, hi = bounds[ch], bounds[ch + 1]
                for h in range(2):
                    rr[k].dma_start(f8t[:, h, :, lo:hi],
                                    f8_d[:, h, :, lo:hi])
                    k += 1

            # zero conv-halo borders + attbd (stt / block copies fill the
            # interior; only rows 0,65 and cols 0,65 must be zero).
            for j in range(4):
                nc.vector.memset(fp[j][:, 0:1, :], 0.0)
                nc.vector.memset(fp[j][:, 65:66, :], 0.0)
                nc.vector.memset(fp[j][:, :, 0:1], 0.0)
                nc.vector.memset(fp[j][:, :, 65:66], 0.0)
                nc.vector.memset(attbd[j], 0.0)

            # ================= Phase 1: transposed Q/K1/K2 + softmax pieces
            # One [128,768] PSUM tile spans 2 banks: Q(0:256)+K1(256:512) in
            # bank A, K2(512:768) in bank B. One start/stop per bank; other
            # first-writes rely on per-element has_written. One ACT does all
            # three exps.
            pv_ctx = tc.tile_pool(name="pv", bufs=3, space="PSUM")
            pv = pv_ctx.__enter__()
            with tc.tile_pool(name="pq", bufs=2, space="PSUM") as pq:

                gk = dict(skip_group_check=True)
                for n in range(NT):
                    pqk = pq.tile([128, 768], F32, tag="qk")
                    psq, psk1, psk2 = pqk[:, 0:256], pqk[:, 256:512], pqk[:, 512:768]
                    for h in range(2):
                        lhsT8 = f8t[:, h, :, n * 128:(n + 1) * 128]
                        nc.tensor.matmul(psq, lhsT8, wq8[:, h],
                                         start=(h == 0), stop=False,
                                         perf_mode=DR, **gk)
                        if h == 0:
                            nc.tensor.matmul(psk1, lhsT8, wk8[:, 0],
                                             start=False, stop=False,
                                             perf_mode=DR, **gk)
                        else:
                            nc.tensor.matmul(psk2, lhsT8, wk8[:, 1],
                                             start=True, stop=True,
                                             perf_mode=DR, **gk)
                    # bias: full-array rank-128 matmul (ones.T @ (bq/128 rows))
                    nc.tensor.matmul(psq, ones, bqr, start=False, stop=True, **gk)

                    act = nc.scalar.activation(qk[:, n, :], pqk, AF.Exp)
                    if n == 2:
                        wv_anchor = act
                    if n == 8:
                        wc_anchor = act
                    if n == 16:
                        fbf_anchor = act

                    # per-head softmax denominator + normalize (in place)
                    q3 = qk[:, n, 0:256].rearrange("p (h e) -> p h e", h=NH)
                    rq = sml.tile([128, NH], F32, tag="rq")
                    nc.vector.tensor_reduce(rq, q3, axis=AX.X, op=ALU.add)
                    rr = sml.tile([128, NH], F32, tag="rr")
                    nc.vector.reciprocal(rr, rq)
                    # alternate engines so Vector doesn't pace the phase
                    meng = nc.vector if n % 2 == 0 else nc.gpsimd
                    meng.tensor_mul(q3, q3, rr.to_broadcast([128, NH, D]))

            # wv/wc: start loading mid-phase-1 (nosync dep keeps them out of
            # the preamble DMA window where F supply is the bottleneck).
            from concourse.tile import add_dep_helper
            d = nc.sync.dma_start(wv8, wv8_d)
            add_dep_helper(d.ins, wv_anchor.ins, sync=False,
                           reason="defer wv load past preamble")
            f_src = [f1_d, f1_d, f2_d, f2_d]
            fb_bounds = [0, 1024, 2048, 3072, 4096]
            fb_rr = [nc.gpsimd, nc.sync, nc.scalar, nc.gpsimd,
                     nc.sync, nc.scalar, nc.gpsimd, nc.sync,
                     nc.gpsimd, nc.sync, nc.gpsimd, nc.sync,
                     nc.gpsimd, nc.sync, nc.gpsimd, nc.sync]
            k = 0
            for ch in range(len(fb_bounds) - 1):
                lo, hi = fb_bounds[ch], fb_bounds[ch + 1]
                for ci in range(4):
                    half = (ci % 2) * 128
                    d = fb_rr[k].dma_start(fbf[:, ci, lo:hi],
                                           f_src[ci][half:half + 128, lo:hi])
                    add_dep_helper(d.ins, fbf_anchor.ins, sync=False,
                                   reason="defer bf16 residual F load past "
                                          "the fp8 phase-1 supply window")
                    k += 1
            d = nc.sync.dma_start(bv, bv_d)
            add_dep_helper(d.ins, wv_anchor.ins, sync=False, reason="defer")
            d = nc.sync.dma_start(muv, muv_d)
            add_dep_helper(d.ins, wv_anchor.ins, sync=False, reason="defer")
            for ci in range(4):
                d = nc.sync.dma_start(wc[:, ci, :, :], wc_d[:, ci, :, :])
                add_dep_helper(d.ins, wc_anchor.ins, sync=False,
                               reason="defer wc load past preamble")
            d = nc.sync.dma_start(bns, bns_d)
            add_dep_helper(d.ins, wc_anchor.ins, sync=False, reason="defer")
            d = nc.sync.dma_start(bnb, bnb_d)
            add_dep_helper(d.ins, wc_anchor.ins, sync=False, reason="defer")

            # ================= Phase 2: V convs + AttT + block-diag
            def emit_v(pv, br, vsb, fci0):
                # evacuation alternates Scalar/Vector: one engine alone
                # (~700ns per [128,512] chunk) would pace the 432ns matmuls
                for m in range(2):
                    for n8 in range(8):
                        psv = pv.tile([128, 512], F32, tag="v",
                                      name=f"psv{br}{m}{n8}")
                        nc.tensor.matmul(
                            psv, wv8[:, br, :, m * 128:(m + 1) * 128],
                            f8t[:, br, :, n8 * 512:(n8 + 1) * 512],
                            start=True, stop=True, perf_mode=DR)
                        dst = vsb[:, m, n8 * 512:(n8 + 1) * 512]
                        if n8 % 2 == 0:
                            nc.scalar.activation(dst, psv, AF.Identity,
                                                 bias=bv[:, br, m:m + 1])
                        else:
                            nc.vector.tensor_scalar_add(dst, psv,
                                                        bv[:, br, m:m + 1])

            emit_v(pv, 0, vsb1, 0)

            scale = {}
            with tc.tile_pool(name="pa", bufs=4, space="PSUM") as pa:
                psa = {}
                for g in range(2):
                    for br in range(2):
                        p = pa.tile([128, 128], F32, tag="a", name=f"psa{br}{g}")
                        psa[(br, g)] = p

                def copy_group(br, g):
                    # diag blocks into the pre-zeroed block-diag tile
                    p = psa[(br, g)]
                    t = attbd[2 * br + g]
                    for hb in range(4):
                        hs = slice(hb * 32, (hb + 1) * 32)
                        nc.any.tensor_copy(t[hs, hs], p[hs, hs])

                def scale_group(br, g):
                    # K-softmax denominators for free: Qs head-rows sum to
                    # 1, so the column sums of each diagonal Att^T block
                    # are exactly S_d = sum_hw exp(K)[hw, d]. One N=1
                    # matmul replaces 32 [128,512] column-sum matmuls.
                    pS = pa.tile([128, 128], F32, tag="a", name=f"ss{br}{g}")
                    pS = pS[:, 0:1]
                    nc.tensor.matmul(pS, attbd[2 * br + g], ones[:, 0:1],
                                     start=True, stop=True)
                    col = sml.tile([128, 1], F32, tag="scat")
                    nc.vector.reciprocal(col, pS)
                    sc = sml.tile([128, 1], F32, tag="scale")
                    nc.vector.tensor_mul(sc, col, muv)   # mu / S_d
                    scale[(br, g)] = sc

                for g in range(2):
                    for n in range(NT):
                        lhsT = qk[:, n, g * 128:(g + 1) * 128]
                        for br in range(2):
                            nc.tensor.matmul(
                                psa[(br, g)], lhsT,
                                qk[:, n, 256 + br * 256 + g * 128:
                                   256 + br * 256 + (g + 1) * 128],
                                start=(n == 0), stop=(n == NT - 1))

                emit_v(pv, 1, vsb2, 2)
                for g in range(2):
                    copy_group(0, g)
                    copy_group(1, g)
                for g in range(2):
                    scale_group(0, g)
                    scale_group(1, g)
            pv_ctx.__exit__(None, None, None)

            # ================= Phase 2b/3: Xw + fused epilogue, interleaved
            # with the conv3x3 output chunks. Xw chunks emit n8-major so all
            # four fp tiles grow top-down together; conv group k (output
            # rows 8k..8k+7) needs epilogue rows n8<=k+1 only. The epilogue
            # alternates Vector/GpSimd so the PE stream never waits on it.
            stt_cnt = 0

            def emit_xw_chunk(px, br, g, n8):
                nonlocal stt_cnt
                vsb, fci0 = ((vsb1, 0), (vsb2, 2))[br]
                pxt = px.tile([128, 512], F32, tag="x")
                nc.tensor.matmul(
                    pxt, attbd[2 * br + g],
                    vsb[:, g, n8 * 512:(n8 + 1) * 512],
                    start=True, stop=True)
                # fused = (Xw_raw * mu/S_d) + F  -> padded layout
                j = 2 * br + g
                out = fp[j][:, 1 + n8 * 8:9 + n8 * 8, 1:65]
                fres = fbf[:, fci0 + g, n8 * 512:(n8 + 1) * 512]
                if stt_cnt % 2 == 0:
                    nc.vector.scalar_tensor_tensor(
                        out=out, in0=pxt, scalar=scale[(br, g)], in1=fres,
                        op0=ALU.mult, op1=ALU.add)
                else:
                    # GPSIMD can't read PSUM: Scalar evacuates+scales, then
                    # GpSimd does the all-SBUF residual add.
                    xt = sml.tile([128, 512], BF16, tag="xt")
                    nc.scalar.activation(xt, pxt, AF.Identity,
                                         scale=scale[(br, g)])
                    nc.gpsimd.tensor_add(out, xt, fres)
                stt_cnt += 1

            def emit_conv_group(pc, kk, m, dma_i):
                pst = pc.tile([128, 512], F32, tag="c", name=f"psc{kk}{m}")
                first, last = (0, 0, 0), (3, 2, 2)
                for ci in range(4):
                    for dy in range(3):
                        for dx in range(3):
                            nc.tensor.matmul(
                                pst, wc[:, ci, (dy * 3 + dx) * 2 + m, :],
                                fp[ci][:, kk * 8 + dy:kk * 8 + dy + 8,
                                       dx:dx + 64],
                                start=((ci, dy, dx) == first),
                                stop=((ci, dy, dx) == last))
                ysb = sml.tile([128, 512], F32, tag="y")
                nc.scalar.activation(ysb, pst, AF.Relu,
                                     bias=bnb[:, m:m + 1],
                                     scale=bns[:, m:m + 1])
                eng = nc.sync if dma_i % 2 == 0 else nc.gpsimd
                eng.dma_start(
                    y_d[m * 128:(m + 1) * 128, kk * 512:(kk + 1) * 512], ysb)

            with tc.tile_pool(name="px", bufs=6, space="PSUM") as px, \
                 tc.tile_pool(name="pc", bufs=2, space="PSUM") as pc:
                conv_done = 0
                for n8 in range(8):
                    for br in range(2):
                        for g in range(2):
                            emit_xw_chunk(px, br, g, n8)
                    # conv group k reads fp rows from epilogue chunks
                    # n8 in {k-1, k, k+1}: emit it only after chunk k+1
                    while conv_done < n8:
                        kk = conv_done
                        emit_conv_group(pc, kk, 0, 2 * kk)
                        emit_conv_group(pc, kk, 1, 2 * kk + 1)
                        conv_done += 1
                while conv_done < 8:
                    kk = conv_done
                    emit_conv_group(pc, kk, 0, 2 * kk)
                    emit_conv_group(pc, kk, 1, 2 * kk + 1)
                    conv_done += 1
    nc.compile()
    return nc


_PROGRAM = None


def _get_program():
    global _PROGRAM
    if _PROGRAM is None:
        _PROGRAM = _build_program()
    return _PROGRAM


def kernel(F1, F2, Wq, bq, Wk1, bk1, Wv1, bv1, Wk2, bk2, Wv2, bv2,
           mu, Wc, gamma, beta, rmean, rvar):
    import os
    import sys
    if "antenv.axon_hooks" not in sys.modules:
        try:
            import antenv.axon_hooks  # noqa: F401
        except ImportError:
            # no profiling hook available: make sure a stray BASS_TRACE
            # can't route run_bass_kernel_spmd into the hook import
            os.environ["BASS_NEVER_TRACE"] = "1"
    f32 = np.float32
    F1 = np.asarray(F1, f32)
    F2 = np.asarray(F2, f32)

    def tile_T(w):   # [O, Cin] -> [128, Cin//128, O] (lhsT tiles)
        wt = np.ascontiguousarray(np.asarray(w, f32).T)      # [Cin, O]
        cin, o = wt.shape
        return wt.reshape(cin // 128, 128, o).transpose(1, 0, 2).astype(_bf)

    def tile_Tf(w):  # like tile_T but fp32 (for fp8 cast)
        wt = np.ascontiguousarray(np.asarray(w, f32).T)
        cin, o = wt.shape
        return wt.reshape(cin // 128, 128, o).transpose(1, 0, 2)

    wq8_h = np.ascontiguousarray(
        tile_Tf(Wq).reshape(128, 2, 2, 256)).astype(_f8)
    wk8_h = np.ascontiguousarray(
        np.stack([tile_Tf(Wk1), tile_Tf(Wk2)], axis=1)).astype(_f8)
    wv8_h = np.ascontiguousarray(
        np.stack([tile_Tf(Wv1), tile_Tf(Wv2)], axis=1)).astype(_f8)

    Wc = np.asarray(Wc, f32)                                 # [256, 512, 3, 3]
    # wc[p, ci, (dy*3+dx)*2+m, col] = Wc[m*128+col, ci*128+p, dy, dx]
    wc_h = Wc.reshape(2, 128, 4, 128, 3, 3)                  # m,col,ci,p,dy,dx
    wc_h = wc_h.transpose(3, 2, 4, 5, 0, 1)                  # p,ci,dy,dx,m,col
    wc_h = np.ascontiguousarray(
        wc_h.reshape(128, 4, 18, 128)).astype(_bf)

    bqr_h = np.ascontiguousarray(
        np.tile((np.asarray(bq, f32) / 128.0).reshape(1, 256),
                (128, 1))).astype(_bf)
    # bv_h[p, br, m] = bv_br[m*128 + p]
    bv_h = np.ascontiguousarray(
        np.stack([np.asarray(bv1, f32), np.asarray(bv2, f32)],
                 axis=0).reshape(2, 2, 128).transpose(2, 0, 1))
    inv = np.asarray(gamma, f32) / np.sqrt(np.asarray(rvar, f32) + BN_EPS)
    b2 = np.asarray(beta, f32) - np.asarray(rmean, f32) * inv
    bns_h = np.ascontiguousarray(inv.reshape(2, 128).T)      # [128, 2]
    bnb_h = np.ascontiguousarray(b2.reshape(2, 128).T)
    muv_h = np.full((128, 1), np.asarray(mu, f32).reshape(-1)[0], f32)

    shared = dict(wq8=wq8_h, wk8=wk8_h, wv8=wv8_h, wc=wc_h, bqr=bqr_h,
                  bv=bv_h, bns=bns_h, bnb=bnb_h, muv=muv_h)

    def f8_pack(b):   # [128, 2(h), 2(ko), HW]: cin = 256h + 128ko + p
        cat = np.concatenate([F1[b].reshape(C, HW), F2[b].reshape(C, HW)])
        return np.ascontiguousarray(
            cat.reshape(2, 2, 128, HW).transpose(2, 0, 1, 3)).astype(_f8)

    in_maps = [dict(f1=np.ascontiguousarray(F1[b].reshape(C, HW)).astype(_bf),
                    f2=np.ascontiguousarray(F2[b].reshape(C, HW)).astype(_bf),
                    f8=f8_pack(b),
                    **shared) for b in range(N_CORES)]

    nc = _get_program()
    res = run_bass_kernel_spmd(nc, in_maps, list(range(N_CORES)))
    kernel.last_results = res

    out = np.stack([res.results[b]["y"] for b in range(N_CORES)])
    return out.reshape(B, C, H, W)


kernel.last_results = None
